# revision 56
# baseline (speedup 1.0000x reference)
"""DeepseekV3 MLA prefill attention on 8 trn2 NeuronCores.

Strategy (single SPMD program, per-core differences live in the input data):
  Phase A: token-split A-projection, feature-major (qkv^T = W_a^T @ h^T).
           m-group order [kpe, q0..q11, kv0..kv3] so the q AllGather parts
           launch early and the kv AllGather (consumed last in phase B)
           launches last. RMS square-sum matmuls are interleaved into the
           NEXT group's MM stream so the PE never stalls on them.
           q-latents are gathered RAW plus a broadcast rstd row (norm is
           applied during phase-B eviction).
  AGs:     q parts 0..2 (part0 carries roped k_pe, part2 carries rstd),
           then kv. All p-major so phase-B loads are contiguous.
  Phase B: Q^T first (packed weights: 3 matmuls per (rp, kq)), per-rp RoPE;
           then K^T / V (their AllGather lands last).
  Phase C: causal attention, software-pipelined with 2-tile lookahead:
           score matmuls for tile ki+2 are emitted before exp/PV of tile
           ki, so the PE never waits for ScalarE's exp. Softmax denominator
           via ones-matmul; broadcast-first reciprocal.
  AG2:     AllGather of attention outputs per head (bf16, feature-major).
  Phase E: column-split o_proj in two K-waves (head-0 strips consumed
           while head-1's AllGather is in flight; bf16 partials + DVE add).
"""

import numpy as np
import ml_dtypes

T = 2048
H = 7168
NH = 16
D_NOPE = 128
D_ROPE = 64
D_V = 128
D_QK = 192
QLR = 1536
KVLR = 512
THETA = 10000.0
EPS = 1e-6
NCORES = 8
TSH = T // NCORES          # 256 tokens per core
HPC = NH // NCORES         # 2 heads per core
WO_COLS = H // NCORES      # 896 output cols per core
NEG = -30000.0             # mask add, enough to zero bf16/f32 exp

BF16 = ml_dtypes.bfloat16

_CACHE = {}


def _build():
    import concourse.mybir as mybir
    import concourse.bacc as bacc

    dt = mybir.dt

    nc = bacc.Bacc(None, target_bir_lowering=False)

    # ---- per-core external inputs (all pre-blocked p-major on host) ----
    hT = nc.declare_dram_parameter("hT", [128, 56 * TSH], dt.bfloat16, isOutput=False)
    wa = nc.declare_dram_parameter("wa", [17 * 56 * 128, 128], dt.bfloat16, isOutput=False)
    wqb = nc.declare_dram_parameter("wqb", [128, 12 * 384], dt.bfloat16, isOutput=False)
    wkvb = nc.declare_dram_parameter("wkvb", [128, 4 * 512], dt.bfloat16, isOutput=False)
    wo = nc.declare_dram_parameter("wo", [128, 16 * WO_COLS], dt.bfloat16, isOutput=False)
    cs_sh = nc.declare_dram_parameter("cs_sh", [64, TSH], dt.bfloat16, isOutput=False)
    cs_full = nc.declare_dram_parameter("cs_full", [64, T], dt.bfloat16, isOutput=False)
    mask1 = nc.declare_dram_parameter("mask1", [128, 128], dt.float32, isOutput=False)
    out = nc.declare_dram_parameter("out", [WO_COLS, T], dt.float32, isOutput=True)

    _build_body(nc, mybir, hT, wa, wqb, wkvb, wo, cs_sh, cs_full, mask1, out)
    nc.compile()
    return nc


def _build_body(nc, mybir, hT, wa, wqb, wkvb, wo, cs_sh, cs_full, mask1, out):
    import concourse.tile as tile
    import contextlib
    dt = mybir.dt
    AF = mybir.ActivationFunctionType
    rg = [list(range(NCORES))]

    with tile.TileContext(nc) as tc:
        top = contextlib.ExitStack()
        with top:
            const = top.enter_context(tc.tile_pool(name="const", bufs=1))
            wpool = top.enter_context(tc.tile_pool(name="wpool", bufs=1))
            epool = top.enter_context(tc.tile_pool(name="ep", bufs=1))
            dram = top.enter_context(tc.tile_pool(name="dram", bufs=1, space="DRAM"))

            ones_b = const.tile([128, 1], dt.bfloat16, tag="ones_b", name="ones_b")
            nc.vector.memset(ones_b[:], 1.0)
            ones_f = const.tile([1, 128], dt.float32, tag="ones_f", name="ones_f")
            nc.vector.memset(ones_f[:], 1.0)
            # only phase-A constants load up front (ACT HWDGE ring, so the
            # SP ring starts streaming h/wa for phase A immediately); B/C/E
            # weights and constants are DMA'd behind phase A's stream
            csc_s = const.tile([32, TSH], dt.bfloat16, tag="csc_s", name="csc_s")
            nc.scalar.dma_start(csc_s[:], cs_sh[0:32, :])
            csn_s = const.tile([32, TSH], dt.bfloat16, tag="csn_s", name="csn_s")
            nc.scalar.dma_start(csn_s[:], cs_sh[32:64, :])

            mask_sb = const.tile([128, 128], dt.float32, tag="mask", name="mask")
            csc_f = const.tile([32, T], dt.bfloat16, tag="csc_f", name="csc_f")
            csn_f = const.tile([32, T], dt.bfloat16, tag="csn_f", name="csn_f")
            wqb_t = wpool.tile([128, 12, 384], dt.bfloat16, tag="wqb", name="wqb")
            wkvb_t = wpool.tile([128, 4, 512], dt.bfloat16, tag="wkvb", name="wkvb")

            # collective buffers, all p-major blocked. part0 is merged:
            # kv 0..3 | kpe | q m0..3; part1: q m4..7; part2: q m8..11 | rstd
            agq_in = [
                dram.tile([128, 9, TSH], dt.bfloat16, tag="agqi0", name="agqi0"),
                dram.tile([128, 4, TSH], dt.bfloat16, tag="agqi1", name="agqi1"),
                dram.tile([128, 5, TSH], dt.bfloat16, tag="agqi2", name="agqi2"),
            ]
            agq_out = [
                dram.tile([NCORES * 128, 9, TSH], dt.bfloat16, tag="agqo0",
                          name="agqo0", addr_space="Shared"),
                dram.tile([NCORES * 128, 4, TSH], dt.bfloat16, tag="agqo1",
                          name="agqo1", addr_space="Shared"),
                dram.tile([NCORES * 128, 5, TSH], dt.bfloat16, tag="agqo2",
                          name="agqo2", addr_space="Shared"),
            ]
            ag2_in = [dram.tile([D_V, T], dt.bfloat16, tag=f"ag2i{h}", name=f"ag2i{h}")
                      for h in range(HPC)]
            ag2_out = [dram.tile([NCORES * D_V, T], dt.bfloat16, tag=f"ag2o{h}",
                                 name=f"ag2o{h}", addr_space="Shared")
                       for h in range(HPC)]

            # ============================================================
            # Phase A: qkv^T = Wa^T @ h^T   [2112, 256] feature-major
            # ============================================================
            with contextlib.ExitStack() as pa:
                h_pool = pa.enter_context(tc.tile_pool(name="h", bufs=1))
                wa_pool = pa.enter_context(tc.tile_pool(name="wa", bufs=8))
                qkv_pool = pa.enter_context(tc.tile_pool(name="qkv", bufs=1))
                x2_pool = pa.enter_context(tc.tile_pool(name="x2", bufs=3))
                agt_pool = pa.enter_context(tc.tile_pool(name="agt", bufs=3))
                ps_a = pa.enter_context(tc.tile_pool(name="ps_a", bufs=4, space="PSUM"))
                ps_ss = pa.enter_context(tc.tile_pool(name="ps_ss", bufs=1, space="PSUM"))
                ps_bc = pa.enter_context(tc.tile_pool(name="ps_bc", bufs=1, space="PSUM"))

                h_all = h_pool.tile([128, 56, TSH], dt.bfloat16, tag="h_all", name="h_all")
                hT3 = hT.rearrange("p (a t) -> p a t", a=56)
                for hh in range(4):
                    nc.sync.dma_start(h_all[:, hh * 14:(hh + 1) * 14, :],
                                      hT3[:, hh * 14:(hh + 1) * 14, :])

                # f32 staging only for the kv groups (normed before AG)
                qkv = [qkv_pool.tile([128, TSH], dt.float32, tag=f"qkv{m}",
                                     name=f"qkv{m}") for m in range(4)]
                kp_raw = qkv_pool.tile([64, TSH], dt.float32, tag="kp_raw", name="kp_raw")
                kp2 = qkv_pool.tile([32, TSH], dt.float32, tag="kp2", name="kp2")

                ss_q = ps_ss.tile([1, TSH], dt.float32, tag="ssq", name="ssq")
                ss_kv = ps_ss.tile([1, TSH], dt.float32, tag="sskv", name="sskv")

                def rstd_bcast(ss, d, name):
                    # [1,T] -> scale+eps -> PE broadcast to 128 partitions ->
                    # full-width DVE reciprocal + ACT sqrt (fast wide ops)
                    ms = x2_pool.tile([1, TSH], dt.float32, tag="ms", name="ms")
                    nc.scalar.activation(ms[:], ss[:], AF.Copy, bias=EPS, scale=1.0 / d)
                    pb = ps_bc.tile([128, TSH], dt.float32, tag=f"bc{name}", name=f"bc{name}")
                    nc.tensor.matmul(pb[:], ones_f[:], ms[:], start=True, stop=True)
                    inv = x2_pool.tile([128, TSH], dt.float32, tag=f"iv{name}", name=f"iv{name}")
                    nc.vector.reciprocal(inv[:], pb[:])
                    rstd = x2_pool.tile([128, TSH], dt.float32, tag=f"rs{name}", name=f"rs{name}")
                    nc.scalar.activation(rstd[:], inv[:], AF.Sqrt)
                    return rstd

                def emit_kv_tail():
                    # ss_kv closed: norm kv latents into the merged part0
                    bc_kv = rstd_bcast(ss_kv, KVLR, "kv")
                    for mm2 in range(4):
                        agt = agt_pool.tile([128, TSH], dt.bfloat16, tag="agt", name="agt")
                        nc.vector.tensor_mul(agt[:], qkv[mm2][:], bc_kv[:])
                        nc.sync.dma_start(agq_in[0][:, mm2, :], agt[:])

                pending_ss = None
                for m in [16, 12, 13, 14, 15] + list(range(12)):
                    mp = 64 if m == 16 else 128
                    # two accumulators (even/odd k) so consecutive matmuls
                    # target different PSUM banks — same-bank back-to-back
                    # accumulation serializes the systolic drain (~25% tax)
                    psA = ps_a.tile([128, TSH], dt.float32, tag="pa", name="pa")
                    psB = ps_a.tile([128, TSH], dt.float32, tag="pa", name="pa")
                    for kc in range(7):
                        chunk = wa_pool.tile([128, 8, 128], dt.bfloat16, tag="wa_c", name="wa_c")
                        r0 = (m * 56 + kc * 8) * 128
                        nc.sync.dma_start(
                            chunk[:],
                            wa[r0:r0 + 1024, :].rearrange("(p a) f -> p a f", a=8),
                        )
                        for k8 in range(8):
                            k = kc * 8 + k8
                            nc.tensor.matmul(
                                (psA if k % 2 == 0 else psB)[:mp, :],
                                chunk[:, k8, :mp],
                                h_all[:, k, :],
                                start=(k < 2),
                                stop=(k >= 54),
                                skip_group_check=True,
                            )
                        if kc == 0 and pending_ss is not None:
                            # previous group's square-sum MM, emitted behind
                            # this group's first chunk so the PE never stalls
                            x2p, ssp, fir, las, was_m = pending_ss
                            nc.tensor.matmul(ssp[:], ones_b[:], x2p[:],
                                             start=fir, stop=las,
                                             skip_group_check=True)
                            pending_ss = None
                            if was_m == 15:
                                emit_kv_tail()
                    if m == 16:
                        nc.scalar.copy(kp_raw[:], psA[:64, :])
                        nc.vector.tensor_add(kp_raw[:], kp_raw[:], psB[:64, :])
                        # move the x2 half to base partition 0 for the DVE ops
                        nc.sync.dma_start(kp2[:], kp_raw[32:64, :])
                        kr1 = agt_pool.tile([32, TSH], dt.bfloat16, tag="kr1", name="kr1")
                        kr2 = agt_pool.tile([32, TSH], dt.bfloat16, tag="kr2", name="kr2")
                        t1 = x2_pool.tile([32, TSH], dt.bfloat16, tag="t1", name="t1")
                        t2 = x2_pool.tile([32, TSH], dt.bfloat16, tag="t2", name="t2")
                        nc.vector.tensor_mul(t1[:], kp_raw[0:32, :], csc_s[:])
                        nc.vector.tensor_mul(t2[:], kp2[:], csn_s[:])
                        nc.vector.tensor_sub(kr1[:], t1[:], t2[:])
                        t3 = x2_pool.tile([32, TSH], dt.bfloat16, tag="t1", name="t1")
                        t4 = x2_pool.tile([32, TSH], dt.bfloat16, tag="t2", name="t2")
                        nc.vector.tensor_mul(t3[:], kp_raw[0:32, :], csn_s[:])
                        nc.vector.tensor_mul(t4[:], kp2[:], csc_s[:])
                        nc.vector.tensor_add(kr2[:], t3[:], t4[:])
                        nc.sync.dma_start(agq_in[0][0:32, 4, :], kr1[:])
                        nc.sync.dma_start(agq_in[0][32:64, 4, :], kr2[:])
                    elif m < 12:
                        # q group: raw bf16 evict straight to the AG buffer
                        agt = agt_pool.tile([128, TSH], dt.bfloat16, tag="agt", name="agt")
                        nc.vector.tensor_copy(agt[:], psA[:])
                        nc.vector.tensor_add(agt[:], agt[:], psB[:])
                        blk = 5 + m if m < 4 else m % 4
                        nc.sync.dma_start(agq_in[m // 4][:, blk, :], agt[:])
                        x2 = x2_pool.tile([128, TSH], dt.bfloat16, tag="x2", name="x2")
                        nc.vector.tensor_mul(x2[:], agt[:], agt[:])
                        pending_ss = (x2, ss_q, m == 0, m == 11, m)
                        if m == 3 or m == 7:
                            nc.gpsimd.collective_compute(
                                "AllGather", mybir.AluOpType.bypass,
                                replica_groups=rg,
                                ins=[agq_in[m // 4].opt()],
                                outs=[agq_out[m // 4].opt()])
                    else:
                        nc.scalar.copy(qkv[m - 12][:], psA[:])
                        nc.vector.tensor_add(qkv[m - 12][:], qkv[m - 12][:], psB[:])
                        x2 = x2_pool.tile([128, TSH], dt.bfloat16, tag="x2", name="x2")
                        nc.vector.tensor_mul(x2[:], qkv[m - 12][:], qkv[m - 12][:])
                        pending_ss = (x2, ss_kv, m == 12, m == 15, m)

                # flush last q square-sum; build rstd row, launch q part2 AG
                x2p, ssp, fir, las, _ = pending_ss
                nc.tensor.matmul(ssp[:], ones_b[:], x2p[:], start=fir, stop=las,
                                 skip_group_check=True)
                bc_q = rstd_bcast(ss_q, QLR, "q")
                brs = agt_pool.tile([128, TSH], dt.bfloat16, tag="brs", name="brs")
                nc.vector.tensor_copy(brs[:], bc_q[:])
                nc.sync.dma_start(agq_in[2][:, 4, :], brs[:])
                nc.gpsimd.collective_compute(
                    "AllGather", mybir.AluOpType.bypass, replica_groups=rg,
                    ins=[agq_in[2].opt()], outs=[agq_out[2].opt()])

            # B/C weights + constants, behind phase A's DMA stream
            nc.scalar.dma_start(wqb_t[:], wqb.rearrange("p (a f) -> p a f", a=12))
            nc.scalar.dma_start(wkvb_t[:], wkvb.rearrange("p (a f) -> p a f", a=4))
            nc.scalar.dma_start(mask_sb[:], mask1[:, :])
            nc.scalar.dma_start(csc_f[:], cs_full[0:32, :])
            nc.scalar.dma_start(csn_f[:], cs_full[32:64, :])

            # ============================================================
            # Phase B: Q^T first (its AGs land first), then K^T / V
            # ============================================================
            bpools = contextlib.ExitStack()
            with bpools:
                act = bpools.enter_context(tc.tile_pool(name="act", bufs=1))
                pbp = contextlib.ExitStack()
                agq_pool = pbp.enter_context(tc.tile_pool(name="agq", bufs=4))
                agkv_pool = pbp.enter_context(tc.tile_pool(name="agkv", bufs=4))
                rsb_pool = pbp.enter_context(tc.tile_pool(name="rsb", bufs=4))
                xx_pool = pbp.enter_context(tc.tile_pool(name="xx", bufs=3))
                rt_pool = pbp.enter_context(tc.tile_pool(name="rt", bufs=1))
                ps_q = pbp.enter_context(tc.tile_pool(name="ps_q", bufs=4, space="PSUM"))
                ps_kt = pbp.enter_context(tc.tile_pool(name="ps_kt", bufs=2, space="PSUM"))
                ps_v = pbp.enter_context(tc.tile_pool(name="ps_v", bufs=2, space="PSUM"))

                qt_n = [act.tile([128, T], dt.bfloat16, tag=f"qtn{h}", name=f"qtn{h}")
                        for h in range(HPC)]
                qraw = act.tile([128, T], dt.bfloat16, tag="qraw", name="qraw")
                qt_r = [act.tile([64, T], dt.bfloat16, tag=f"qtr{h}", name=f"qtr{h}")
                        for h in range(HPC)]
                kt_n = [act.tile([128, T], dt.bfloat16, tag=f"ktn{h}", name=f"ktn{h}")
                        for h in range(HPC)]
                kpe_t = act.tile([64, T], dt.bfloat16, tag="kpet", name="kpet")
                # v2_t[ti]: both heads' V for token tile ti, cols h*128..
                v2_t = [act.tile([128, 2 * D_V], dt.bfloat16, tag=f"v{i}", name=f"v{i}")
                        for i in range(16)]

                # --- all B loads issued up front: kv (lands first), then
                # aq parts 0/1, then part 2 last so nothing it waits on can
                # head-of-line-block earlier loads; rstd/kpe on the ACT ring
                akv_all = []
                aq_all = []
                rstd_all = []
                for rp in range(4):
                    akv = agkv_pool.tile([128, 4, 512], dt.bfloat16, tag="akv", name="akv")
                    akv_all.append(akv)
                    for s in range(2):
                        r = 2 * rp + s
                        nc.sync.dma_start(
                            akv[:, :, s * TSH:(s + 1) * TSH],
                            agq_out[0][r * 128:(r + 1) * 128, 0:4, :])
                for rp in range(4):
                    aq_all.append(agq_pool.tile([128, 12, 512], dt.bfloat16,
                                                tag="aq", name="aq"))
                    rstd_all.append(rsb_pool.tile([128, 512], dt.bfloat16,
                                                  tag="rsb", name="rsb"))
                for p3 in range(3):
                    for rp in range(4):
                        for s in range(2):
                            r = 2 * rp + s
                            ssl = slice(s * TSH, (s + 1) * TSH)
                            srcblk = 5 if p3 == 0 else 0
                            nc.sync.dma_start(
                                aq_all[rp][:, p3 * 4:p3 * 4 + 4, ssl],
                                agq_out[p3][r * 128:(r + 1) * 128,
                                            srcblk:srcblk + 4, :])
                for rp in range(4):
                    for s in range(2):
                        r = 2 * rp + s
                        ssl = slice(s * TSH, (s + 1) * TSH)
                        nc.scalar.dma_start(
                            rstd_all[rp][:, ssl],
                            agq_out[2][r * 128:(r + 1) * 128, 4, :])
                        nc.scalar.dma_start(
                            kpe_t[:, r * TSH:(r + 1) * TSH],
                            agq_out[0][r * 128:r * 128 + 64, 4, :])

                # --- KV path first (its AG lands first) ---
                for rp in range(4):
                    tsl = slice(rp * 512, (rp + 1) * 512)
                    akv = akv_all[rp]
                    pk0 = ps_kt.tile([128, 512], dt.float32, tag="pkv", name="pkv")
                    pk1 = ps_kt.tile([128, 512], dt.float32, tag="pkv", name="pkv")
                    for kk in range(4):
                        nc.tensor.matmul(
                            pk0[:], wkvb_t[:, kk, 0:128], akv[:, kk, :],
                            start=(kk == 0), stop=(kk == 3),
                            skip_group_check=True)
                        nc.tensor.matmul(
                            pk1[:], wkvb_t[:, kk, 128:256], akv[:, kk, :],
                            start=(kk == 0), stop=(kk == 3),
                            skip_group_check=True)
                    nc.vector.tensor_copy(kt_n[0][:, tsl], pk0[:])
                    nc.vector.tensor_copy(kt_n[1][:, tsl], pk1[:])
                    for pr in range(2):
                        pv0 = ps_v.tile([128, 2 * D_V], dt.float32, tag="pv_b", name="pv_b")
                        pv1 = ps_v.tile([128, 2 * D_V], dt.float32, tag="pv_b", name="pv_b")
                        for kk in range(4):
                            for s4, pv in ((2 * pr, pv0), (2 * pr + 1, pv1)):
                                nc.tensor.matmul(
                                    pv[:],
                                    akv[:, kk, s4 * 128:(s4 + 1) * 128],
                                    wkvb_t[:, kk, 256:512],
                                    start=(kk == 0), stop=(kk == 3),
                                    skip_group_check=True)
                        nc.vector.tensor_copy(v2_t[4 * rp + 2 * pr][:], pv0[:])
                        nc.vector.tensor_copy(v2_t[4 * rp + 2 * pr + 1][:], pv1[:])

                # --- Q path ---
                for rp in range(4):
                    tsl = slice(rp * 512, (rp + 1) * 512)
                    aq = aq_all[rp]
                    rstd_bc = rstd_all[rp]
                    pnA = ps_q.tile([128, 512], dt.float32, tag="pq", name="pq")
                    pnB = ps_q.tile([128, 512], dt.float32, tag="pq", name="pq")
                    pR = ps_q.tile([128, 512], dt.float32, tag="pq", name="pq")
                    for kq in range(12):
                        nc.tensor.matmul(
                            pnA[:], wqb_t[:, kq, 0:128], aq[:, kq, :],
                            start=(kq == 0), stop=(kq == 11), skip_group_check=True)
                        nc.tensor.matmul(
                            pnB[:], wqb_t[:, kq, 128:256], aq[:, kq, :],
                            start=(kq == 0), stop=(kq == 11), skip_group_check=True)
                        nc.tensor.matmul(
                            pR[:], wqb_t[:, kq, 256:384], aq[:, kq, :],
                            start=(kq == 0), stop=(kq == 11), skip_group_check=True)
                    nc.vector.tensor_mul(qt_n[0][:, tsl], pnA[:], rstd_bc[:])
                    nc.vector.tensor_mul(qt_n[1][:, tsl], pnB[:], rstd_bc[:])
                    nc.vector.tensor_mul(qraw[:, tsl], pR[:], rstd_bc[:])
                    # rope: qraw rows [h0ev, h0od, h1ev, h1od] (32 each)
                    x2h0 = xx_pool.tile([32, 512], dt.bfloat16, tag="xx", name="xx")
                    x1h1 = xx_pool.tile([32, 512], dt.bfloat16, tag="xx", name="xx")
                    x2h1 = xx_pool.tile([32, 512], dt.bfloat16, tag="xx", name="xx")
                    nc.sync.dma_start(x2h0[:], qraw[32:64, tsl])
                    nc.sync.dma_start(x1h1[:], qraw[64:96, tsl])
                    nc.sync.dma_start(x2h1[:], qraw[96:128, tsl])
                    for h, (x1, x2) in enumerate([(qraw[0:32, tsl], x2h0[:]),
                                                  (x1h1[:], x2h1[:])]):
                        cc = csc_f[:, tsl]
                        ss = csn_f[:, tsl]
                        t1 = rt_pool.tile([32, 512], dt.bfloat16, tag="qt1", name="qt1")
                        t2 = rt_pool.tile([32, 512], dt.bfloat16, tag="qt2", name="qt2")
                        nc.vector.tensor_mul(t1[:], x1, cc)
                        nc.vector.tensor_mul(t2[:], x2, ss)
                        nc.vector.tensor_sub(qt_r[h][0:32, tsl], t1[:], t2[:])
                        t3 = rt_pool.tile([32, 512], dt.bfloat16, tag="qt1", name="qt1")
                        t4 = rt_pool.tile([32, 512], dt.bfloat16, tag="qt2", name="qt2")
                        nc.vector.tensor_mul(t3[:], x1, ss)
                        nc.vector.tensor_mul(t4[:], x2, cc)
                        r2t = rt_pool.tile([32, 512], dt.bfloat16, tag="r2t", name="r2t")
                        nc.vector.tensor_add(r2t[:], t3[:], t4[:])
                        nc.sync.dma_start(qt_r[h][32:64, tsl], r2t[:])

                pbp.close()

                # ========================================================
                # Phase C: attention, software-pipelined (lookahead 2)
                # ========================================================
                with contextlib.ExitStack() as pc:
                    pt_pool = pc.enter_context(tc.tile_pool(name="pt", bufs=4))
                    sm_pool = pc.enter_context(tc.tile_pool(name="sm", bufs=2))
                    acc_pool = pc.enter_context(tc.tile_pool(name="acc", bufs=2))
                    ps_s = pc.enter_context(tc.tile_pool(name="ps_s", bufs=3, space="PSUM"))
                    ps_pv = pc.enter_context(tc.tile_pool(name="ps_pv", bufs=2, space="PSUM"))
                    ps_l = pc.enter_context(tc.tile_pool(name="ps_l", bufs=2, space="PSUM"))
                    ps_b = pc.enter_context(tc.tile_pool(name="ps_b", bufs=1, space="PSUM"))

                    attn = [act.tile([128, T], dt.bfloat16, tag=f"attn{h}",
                                     name=f"attn{h}") for h in range(HPC)]
                    # head-0 strips live in a top-level pool so their loads
                    # can be issued as soon as the first AG2 completes; the
                    # o_proj weights load here too (phase-B pools just freed)
                    stripsA = [epool.tile([128, 2, T], dt.bfloat16, tag=f"sa{p}",
                                          name=f"sa{p}") for p in range(4)]
                    wo_t = epool.tile([128, 16, WO_COLS], dt.bfloat16, tag="wo",
                                      name="wo")
                    nc.scalar.dma_start(wo_t[:], wo.rearrange("p (a f) -> p a f", a=16))

                    # interleave the two heads' tile streams (head 0 one
                    # j-chunk ahead): whenever one head is in its serial
                    # j-end normalization chain, the PE streams the other
                    # head's independent score/PV matmuls
                    steps0 = [(0, j, ki) for j in range(4) for ki in range(4 * j + 4)]
                    steps1 = [(1, j, ki) for j in range(4) for ki in range(4 * j + 4)]
                    steps = steps0[:12]
                    i0, i1 = 12, 0
                    while i0 < len(steps0) or i1 < len(steps1):
                        if i0 < len(steps0):
                            steps.append(steps0[i0]); i0 += 1
                        if i1 < len(steps1):
                            steps.append(steps1[i1]); i1 += 1

                    def emit_s1(h, j, ki):
                        # nope-dim score matmul (start of the 192-dim pair)
                        off = max(0, (ki - 4 * j) * 128)
                        w = 512 - off
                        qs0 = j * 512 + off
                        ksl = slice(ki * 128, (ki + 1) * 128)
                        ps = ps_s.tile([128, 512], dt.float32, tag="ps", name="ps")
                        nc.tensor.matmul(
                            ps[:, off:], kt_n[h][:, ksl], qt_n[h][:, qs0:qs0 + w],
                            start=True, stop=False, skip_group_check=True)
                        return ps

                    def emit_s2(h, j, ki, ps):
                        # rope-dim score matmul; staggered one step behind
                        # s1 so consecutive matmuls alternate PSUM banks
                        off = max(0, (ki - 4 * j) * 128)
                        w = 512 - off
                        qs0 = j * 512 + off
                        ksl = slice(ki * 128, (ki + 1) * 128)
                        nc.tensor.matmul(
                            ps[:, off:], kpe_t[:, ksl], qt_r[h][:, qs0:qs0 + w],
                            start=False, stop=True, skip_group_check=True)

                    pend = {}
                    pend[steps[0]] = emit_s1(*steps[0])
                    pend[steps[1]] = emit_s1(*steps[1])
                    emit_s2(*steps[0], pend[steps[0]])
                    cur = {0: {}, 1: {}}
                    for idx, (h, j, ki) in enumerate(steps):
                        nk = 4 * j + 4
                        qsl = slice(j * 512, (j + 1) * 512)
                        if ki == 0:
                            cur[h]['ppv'] = ps_pv.tile([128, 512], dt.float32,
                                                       tag="ppv", name="ppv")
                            cur[h]['acc'] = acc_pool.tile([128, 512], dt.bfloat16,
                                                          tag="acc", name="acc")
                        if idx + 2 < len(steps):
                            pend[steps[idx + 2]] = emit_s1(*steps[idx + 2])
                        if idx + 1 < len(steps):
                            emit_s2(*steps[idx + 1], pend[steps[idx + 1]])
                        ps = pend.pop((h, j, ki))
                        off = max(0, (ki - 4 * j) * 128)
                        if ki >= 4 * j:
                            nc.vector.tensor_add(
                                ps[:, off:off + 128], ps[:, off:off + 128],
                                mask_sb[:])
                        pt = pt_pool.tile([128, 512], dt.bfloat16, tag="pt", name="pt")
                        nc.scalar.activation(pt[:, off:], ps[:, off:], AF.Exp)
                        # denominator partial sums on DVE (saves a PE matmul
                        # per tile); reduced across partitions once per j
                        if ki == 0:
                            nc.vector.tensor_copy(cur[h]['acc'][:], pt[:])
                        else:
                            nc.vector.tensor_add(cur[h]['acc'][:, off:],
                                                 cur[h]['acc'][:, off:], pt[:, off:])
                        nc.tensor.matmul(
                            cur[h]['ppv'][:, off:], v2_t[ki][:, h * D_V:(h + 1) * D_V],
                            pt[:, off:],
                            start=(ki == 0), stop=(ki == nk - 1),
                            skip_group_check=True)
                        if ki == nk - 1:
                            # l = colsum(acc); 1/l via exp(-ln(l)) on ACT
                            # (DVE reciprocal is ~6ns/elem; ln+exp is ~1.7x
                            # faster and both live in the loaded table set)
                            pl = ps_l.tile([1, 512], dt.float32, tag="pl", name="pl")
                            nc.tensor.matmul(pl[:], ones_b[:], cur[h]['acc'][:],
                                             start=True, stop=True,
                                             skip_group_check=True)
                            pls = sm_pool.tile([1, 512], dt.float32, tag="pls", name="pls")
                            nc.scalar.copy(pls[:], pl[:])
                            pb = ps_b.tile([128, 512], dt.float32, tag="pb", name="pb")
                            nc.tensor.matmul(pb[:], ones_f[:], pls[:],
                                             start=True, stop=True,
                                             skip_group_check=True)
                            # NOTE: keep everything here off ScalarE's Exp
                            # table set — Ln/Reciprocal live in different
                            # sets and each switch costs ~2.7us + disrupts
                            # the exp pipeline (measured 9 table loads/C)
                            rb = sm_pool.tile([128, 512], dt.float32, tag="rb", name="rb")
                            nc.vector.reciprocal(rb[:], pb[:])
                            nc.vector.tensor_mul(attn[h][:, qsl], cur[h]['ppv'][:], rb[:])
                            if j == 3:
                                nc.sync.dma_start(ag2_in[h][:], attn[h][:])
                                nc.gpsimd.collective_compute(
                                    "AllGather", mybir.AluOpType.bypass,
                                    replica_groups=rg,
                                    ins=[ag2_in[h].opt()], outs=[ag2_out[h].opt()])
                                if h == 0:
                                    for p in range(4):
                                        for z in range(2):
                                            r = 2 * p + z
                                            nc.sync.dma_start(
                                                stripsA[p][:, z, :],
                                                ag2_out[0][r * 128:(r + 1) * 128, :])

            # ============================================================
            # Phase E: o_proj column slice, two K-waves (head0 then head1)
            # ============================================================
            with contextlib.ExitStack() as pe:
                ao_pool = pe.enter_context(tc.tile_pool(name="ao", bufs=1))
                oa_pool = pe.enter_context(tc.tile_pool(name="oa", bufs=1))
                oo_pool = pe.enter_context(tc.tile_pool(name="oo", bufs=3))
                ps_o = pe.enter_context(tc.tile_pool(name="ps_o", bufs=8, space="PSUM"))

                # strips paired [128, 2, T]; wo blocks host-reordered
                # [0,2,..,14, 1,3,..,15] (wave A then wave B); stripsA loads
                # were issued at the first AG2 trigger in phase C, stripsB
                # loads here overlap wave A's matmuls
                stripsB = [ao_pool.tile([128, 2, T], dt.bfloat16, tag=f"sb{p}",
                                        name=f"sb{p}") for p in range(4)]
                otA = [oa_pool.tile([128, T], dt.bfloat16, tag=f"oa{mt}", name=f"oa{mt}")
                       for mt in range(7)]
                for p in range(4):
                    for z in range(2):
                        r = 2 * p + z
                        nc.sync.dma_start(stripsB[p][:, z, :],
                                          ag2_out[1][r * 128:(r + 1) * 128, :])
                # wave A: heads 0,2,..,14 (wo blocks 0..7)
                for mt in range(7):
                    msl = slice(mt * 128, (mt + 1) * 128)
                    pes = [ps_o.tile([128, 512], dt.float32, tag="po", name="po")
                           for _ in range(4)]
                    for p in range(4):
                        for z in range(2):
                            for n in range(4):
                                nc.tensor.matmul(
                                    pes[n][:], wo_t[:, 2 * p + z, msl],
                                    stripsA[p][:, z, n * 512:(n + 1) * 512],
                                    start=(p == 0 and z == 0),
                                    stop=(p == 3 and z == 1),
                                    skip_group_check=True)
                    for n in range(4):
                        nc.scalar.copy(otA[mt][:, n * 512:(n + 1) * 512], pes[n][:])
                # wave B: heads 1,3,..,15 (wo blocks 8..15), add wave A partial
                for mt in range(7):
                    msl = slice(mt * 128, (mt + 1) * 128)
                    pes = [ps_o.tile([128, 512], dt.float32, tag="po", name="po")
                           for _ in range(4)]
                    for p in range(4):
                        for z in range(2):
                            for n in range(4):
                                nc.tensor.matmul(
                                    pes[n][:], wo_t[:, 8 + 2 * p + z, msl],
                                    stripsB[p][:, z, n * 512:(n + 1) * 512],
                                    start=(p == 0 and z == 0),
                                    stop=(p == 3 and z == 1),
                                    skip_group_check=True)
                    ot = oo_pool.tile([128, T], dt.float32, tag="ot", name="ot")
                    for n in range(4):
                        nc.vector.tensor_add(
                            ot[:, n * 512:(n + 1) * 512], pes[n][:],
                            otA[mt][:, n * 512:(n + 1) * 512])
                    nc.sync.dma_start(out[msl, :], ot[:])


def _prep_inputs(hidden_states, positions, W_qkv_a, gamma_q, W_qb, gamma_kv,
                 W_kvb, W_o):
    f32 = np.float32
    perm = np.concatenate([np.arange(0, D_ROPE, 2), np.arange(1, D_ROPE, 2)])
    scale = np.float32(D_QK ** -0.5)

    def pmajor(w, nblk):
        # [nblk*128, F] -> [128, nblk*F] so a partition line is contiguous
        F = w.shape[1]
        return np.ascontiguousarray(
            w.reshape(nblk, 128, F).transpose(1, 0, 2).reshape(128, nblk * F)
        ).astype(BF16)

    # A-projection weights: de-interleave k_pe output cols, block layout
    Wa = np.asarray(W_qkv_a, f32).copy()
    Wa[:, QLR + KVLR:] = Wa[:, QLR + KVLR:][:, perm]
    Wa = np.concatenate([Wa, np.zeros((H, 64), f32)], axis=1)  # pad 2112->2176
    wa_b = (
        Wa.reshape(7, 8, 128, 17, 128)   # [kc, k8, p, m, f]
        .transpose(3, 0, 2, 1, 4)        # [m, kc, p, k8, f]
        .reshape(17 * 56 * 128, 128)
        .astype(BF16)
    )

    # q_b weights: fold gamma_q and score scale, de-interleave rope cols,
    # pack per-core as [h0 nope | h1 nope | h0 ev | h0 od | h1 ev | h1 od]
    Wqb = (np.asarray(W_qb, f32) * np.asarray(gamma_q, f32)[:, None] * scale)
    Wqb = Wqb.reshape(QLR, NH, D_QK)
    Wqb_n = Wqb[:, :, :D_NOPE]
    Wqb_r = Wqb[:, :, D_NOPE:][:, :, perm]   # [QLR, NH, 64] ev|od

    # kv_b weights: fold gamma_kv
    Wkvb = (np.asarray(W_kvb, f32) * np.asarray(gamma_kv, f32)[:, None])
    Wkvb = Wkvb.reshape(KVLR, NH, D_NOPE + D_V)

    Wo = np.asarray(W_o, f32)

    hTf = np.asarray(hidden_states, f32).T.astype(BF16)  # [H, T]

    pos = np.asarray(positions, f32)
    inv_freq = 1.0 / (THETA ** (np.arange(D_ROPE // 2, dtype=f32) / (D_ROPE // 2)))
    freqs = pos[:, None] * inv_freq[None, :]          # [T, 32]
    cos = np.cos(freqs).astype(f32).T                 # [32, T]
    sin = np.sin(freqs).astype(f32).T
    cs = np.concatenate([cos, sin], axis=0)           # [64, T]

    kk = np.arange(128)[:, None]
    qq = np.arange(128)[None, :]
    mask1 = np.where(qq < kk, np.float32(NEG), np.float32(0.0)).astype(f32)

    in_maps = []
    for c in range(NCORES):
        h0, h1 = 2 * c, 2 * c + 1
        wqb_c = np.concatenate(
            [Wqb_n[:, h0, :], Wqb_n[:, h1, :], Wqb_r[:, h0, :], Wqb_r[:, h1, :]],
            axis=1)                                  # [QLR, 384]
        wkvb_c = np.concatenate(
            [Wkvb[:, h0, :D_NOPE], Wkvb[:, h1, :D_NOPE],
             Wkvb[:, h0, D_NOPE:], Wkvb[:, h1, D_NOPE:]], axis=1)  # [KVLR, 512]
        hsh = np.ascontiguousarray(hTf[:, c * TSH:(c + 1) * TSH])  # [H, TSH]
        hsh = hsh.reshape(56, 128, TSH).transpose(1, 0, 2).reshape(128, 56 * TSH)
        # wo: fp8, head blocks reordered [0,2,..,14, 1,3,..,15] so the
        # DoubleRow pairs within each AG2 wave are adjacent
        wo_c = Wo[:, c * WO_COLS:(c + 1) * WO_COLS].reshape(16, 128, WO_COLS)
        wo_c = wo_c[list(range(0, 16, 2)) + list(range(1, 16, 2))]
        wo_c = (wo_c.transpose(1, 0, 2).reshape(128, 16 * WO_COLS)
                .astype(BF16))
        in_maps.append({
            "hT": np.ascontiguousarray(hsh),
            "wa": wa_b,
            "wqb": pmajor(wqb_c, 12),
            "wkvb": pmajor(wkvb_c, 4),
            "wo": np.ascontiguousarray(wo_c),
            "cs_sh": np.ascontiguousarray(cs[:, c * TSH:(c + 1) * TSH]).astype(BF16),
            "cs_full": cs.astype(BF16),
            "mask1": mask1,
        })
    return in_maps


def kernel(hidden_states, positions, W_qkv_a, gamma_q, W_qb, gamma_kv, W_kvb,
           W_o, _trace=False, _tmpdir=None):
    from concourse.bass_utils import run_bass_kernel_spmd

    if "nc" not in _CACHE:
        _CACHE["nc"] = _build()
    nc = _CACHE["nc"]

    in_maps = _prep_inputs(hidden_states, positions, W_qkv_a, gamma_q, W_qb,
                           gamma_kv, W_kvb, W_o)
    res = run_bass_kernel_spmd(nc, in_maps, list(range(NCORES)), trace=_trace,
                               tmpdir=_tmpdir)
    _CACHE["last_result"] = res
    out = np.concatenate(
        [res.results[c]["out"].T for c in range(NCORES)], axis=1)
    return out.astype(np.float32)


# revision 57
# speedup vs baseline: 1.0707x; 1.0707x over previous
"""DeepseekV3 MLA prefill attention on 8 trn2 NeuronCores.

Strategy (single SPMD program, per-core differences live in the input data):
  Phase A: token-split A-projection, feature-major (qkv^T = W_a^T @ h^T).
           m-group order [kpe, q0..q11, kv0..kv3] so the q AllGather parts
           launch early and the kv AllGather (consumed last in phase B)
           launches last. RMS square-sum matmuls are interleaved into the
           NEXT group's MM stream so the PE never stalls on them.
           q-latents are gathered RAW plus a broadcast rstd row (norm is
           applied during phase-B eviction).
  AGs:     q parts 0..2 (part0 carries roped k_pe, part2 carries rstd),
           then kv. All p-major so phase-B loads are contiguous.
  Phase B: Q^T first (packed weights: 3 matmuls per (rp, kq)), per-rp RoPE;
           then K^T / V (their AllGather lands last).
  Phase C: causal attention, software-pipelined with 2-tile lookahead:
           score matmuls for tile ki+2 are emitted before exp/PV of tile
           ki, so the PE never waits for ScalarE's exp. Softmax denominator
           via ones-matmul; broadcast-first reciprocal.
  AG2:     AllGather of attention outputs per head (bf16, feature-major).
  Phase E: column-split o_proj in two K-waves (head-0 strips consumed
           while head-1's AllGather is in flight; bf16 partials + DVE add).
"""

import numpy as np
import ml_dtypes

T = 2048
H = 7168
NH = 16
D_NOPE = 128
D_ROPE = 64
D_V = 128
D_QK = 192
QLR = 1536
KVLR = 512
THETA = 10000.0
EPS = 1e-6
NCORES = 8
TSH = T // NCORES          # 256 tokens per core
HPC = NH // NCORES         # 2 heads per core
WO_COLS = H // NCORES      # 896 output cols per core
NEG = -30000.0             # mask add, enough to zero bf16/f32 exp

BF16 = ml_dtypes.bfloat16

_CACHE = {}


def _build():
    import concourse.mybir as mybir
    import concourse.bacc as bacc

    dt = mybir.dt

    nc = bacc.Bacc(None, target_bir_lowering=False)

    # ---- per-core external inputs (all pre-blocked p-major on host) ----
    hT = nc.declare_dram_parameter("hT", [128, 56 * TSH], dt.bfloat16, isOutput=False)
    wa = nc.declare_dram_parameter("wa", [17 * 56 * 128, 128], dt.bfloat16, isOutput=False)
    wqb = nc.declare_dram_parameter("wqb", [128, 12 * 384], dt.bfloat16, isOutput=False)
    wkvb = nc.declare_dram_parameter("wkvb", [128, 4 * 512], dt.bfloat16, isOutput=False)
    wo = nc.declare_dram_parameter("wo", [128, 16 * WO_COLS], dt.bfloat16, isOutput=False)
    cs_sh = nc.declare_dram_parameter("cs_sh", [64, TSH], dt.bfloat16, isOutput=False)
    cs_full = nc.declare_dram_parameter("cs_full", [64, T], dt.bfloat16, isOutput=False)
    mask1 = nc.declare_dram_parameter("mask1", [128, 128], dt.float32, isOutput=False)
    out = nc.declare_dram_parameter("out", [WO_COLS, T], dt.float32, isOutput=True)

    _build_body(nc, mybir, hT, wa, wqb, wkvb, wo, cs_sh, cs_full, mask1, out)
    nc.compile()
    return nc


def _build_body(nc, mybir, hT, wa, wqb, wkvb, wo, cs_sh, cs_full, mask1, out):
    import concourse.tile as tile
    import contextlib
    dt = mybir.dt
    AF = mybir.ActivationFunctionType
    rg = [list(range(NCORES))]

    with tile.TileContext(nc) as tc:
        top = contextlib.ExitStack()
        with top:
            const = top.enter_context(tc.tile_pool(name="const", bufs=1))
            wpool = top.enter_context(tc.tile_pool(name="wpool", bufs=1))
            epool = top.enter_context(tc.tile_pool(name="ep", bufs=1))
            dram = top.enter_context(tc.tile_pool(name="dram", bufs=1, space="DRAM"))

            ones_b = const.tile([128, 1], dt.bfloat16, tag="ones_b", name="ones_b")
            nc.vector.memset(ones_b[:], 1.0)
            ones_f = const.tile([1, 128], dt.float32, tag="ones_f", name="ones_f")
            nc.vector.memset(ones_f[:], 1.0)
            # only phase-A constants load up front (ACT HWDGE ring, so the
            # SP ring starts streaming h/wa for phase A immediately); B/C/E
            # weights and constants are DMA'd behind phase A's stream
            csc_s = const.tile([32, TSH], dt.bfloat16, tag="csc_s", name="csc_s")
            nc.scalar.dma_start(csc_s[:], cs_sh[0:32, :])
            csn_s = const.tile([32, TSH], dt.bfloat16, tag="csn_s", name="csn_s")
            nc.scalar.dma_start(csn_s[:], cs_sh[32:64, :])

            mask_sb = const.tile([128, 128], dt.float32, tag="mask", name="mask")
            csc_f = const.tile([32, T], dt.bfloat16, tag="csc_f", name="csc_f")
            csn_f = const.tile([32, T], dt.bfloat16, tag="csn_f", name="csn_f")
            wqb_t = wpool.tile([128, 12, 384], dt.bfloat16, tag="wqb", name="wqb")
            wkvb_t = wpool.tile([128, 4, 512], dt.bfloat16, tag="wkvb", name="wkvb")

            # collective buffers, all p-major blocked. part0 is merged:
            # kv 0..3 | kpe | q m0..3; part1: q m4..7; part2: q m8..11 | rstd
            agq_in = [
                dram.tile([128, 9, TSH], dt.bfloat16, tag="agqi0", name="agqi0"),
                dram.tile([128, 4, TSH], dt.bfloat16, tag="agqi1", name="agqi1"),
                dram.tile([128, 5, TSH], dt.bfloat16, tag="agqi2", name="agqi2"),
            ]
            agq_out = [
                dram.tile([NCORES * 128, 9, TSH], dt.bfloat16, tag="agqo0",
                          name="agqo0", addr_space="Shared"),
                dram.tile([NCORES * 128, 4, TSH], dt.bfloat16, tag="agqo1",
                          name="agqo1", addr_space="Shared"),
                dram.tile([NCORES * 128, 5, TSH], dt.bfloat16, tag="agqo2",
                          name="agqo2", addr_space="Shared"),
            ]
            ag2_in = [dram.tile([D_V, T], dt.bfloat16, tag=f"ag2i{h}", name=f"ag2i{h}")
                      for h in range(HPC)]
            ag2_out = [dram.tile([NCORES * D_V, T], dt.bfloat16, tag=f"ag2o{h}",
                                 name=f"ag2o{h}", addr_space="Shared")
                       for h in range(HPC)]

            # ============================================================
            # Phase A: qkv^T = Wa^T @ h^T   [2112, 256] feature-major
            # ============================================================
            with contextlib.ExitStack() as pa:
                h_pool = pa.enter_context(tc.tile_pool(name="h", bufs=1))
                wa_pool = pa.enter_context(tc.tile_pool(name="wa", bufs=8))
                qkv_pool = pa.enter_context(tc.tile_pool(name="qkv", bufs=1))
                x2_pool = pa.enter_context(tc.tile_pool(name="x2", bufs=3))
                agt_pool = pa.enter_context(tc.tile_pool(name="agt", bufs=3))
                ps_a = pa.enter_context(tc.tile_pool(name="ps_a", bufs=4, space="PSUM"))
                ps_ss = pa.enter_context(tc.tile_pool(name="ps_ss", bufs=1, space="PSUM"))
                ps_bc = pa.enter_context(tc.tile_pool(name="ps_bc", bufs=1, space="PSUM"))

                h_all = h_pool.tile([128, 56, TSH], dt.bfloat16, tag="h_all", name="h_all")
                hT3 = hT.rearrange("p (a t) -> p a t", a=56)
                for hh in range(4):
                    nc.sync.dma_start(h_all[:, hh * 14:(hh + 1) * 14, :],
                                      hT3[:, hh * 14:(hh + 1) * 14, :])

                # f32 staging only for the kv groups (normed before AG)
                qkv = [qkv_pool.tile([128, TSH], dt.float32, tag=f"qkv{m}",
                                     name=f"qkv{m}") for m in range(4)]
                kp_raw = qkv_pool.tile([64, TSH], dt.float32, tag="kp_raw", name="kp_raw")
                kp2 = qkv_pool.tile([32, TSH], dt.float32, tag="kp2", name="kp2")

                ss_q = ps_ss.tile([1, TSH], dt.float32, tag="ssq", name="ssq")
                ss_kv = ps_ss.tile([1, TSH], dt.float32, tag="sskv", name="sskv")

                def rstd_bcast(ss, d, name):
                    # [1,T] -> scale+eps -> PE broadcast to 128 partitions ->
                    # full-width DVE reciprocal + ACT sqrt (fast wide ops)
                    ms = x2_pool.tile([1, TSH], dt.float32, tag="ms", name="ms")
                    nc.scalar.activation(ms[:], ss[:], AF.Copy, bias=EPS, scale=1.0 / d)
                    pb = ps_bc.tile([128, TSH], dt.float32, tag=f"bc{name}", name=f"bc{name}")
                    nc.tensor.matmul(pb[:], ones_f[:], ms[:], start=True, stop=True)
                    inv = x2_pool.tile([128, TSH], dt.float32, tag=f"iv{name}", name=f"iv{name}")
                    nc.vector.reciprocal(inv[:], pb[:])
                    rstd = x2_pool.tile([128, TSH], dt.float32, tag=f"rs{name}", name=f"rs{name}")
                    nc.scalar.activation(rstd[:], inv[:], AF.Sqrt)
                    return rstd

                def emit_kv_tail():
                    # ss_kv closed: norm kv latents into the merged part0
                    bc_kv = rstd_bcast(ss_kv, KVLR, "kv")
                    for mm2 in range(4):
                        agt = agt_pool.tile([128, TSH], dt.bfloat16, tag="agt", name="agt")
                        nc.vector.tensor_mul(agt[:], qkv[mm2][:], bc_kv[:])
                        nc.sync.dma_start(agq_in[0][:, mm2, :], agt[:])

                pending_ss = None
                for m in [16, 12, 13, 14, 15] + list(range(12)):
                    mp = 64 if m == 16 else 128
                    # two accumulators (even/odd k) so consecutive matmuls
                    # target different PSUM banks — same-bank back-to-back
                    # accumulation serializes the systolic drain (~25% tax)
                    psA = ps_a.tile([128, TSH], dt.float32, tag="pa", name="pa")
                    psB = ps_a.tile([128, TSH], dt.float32, tag="pa", name="pa")
                    for kc in range(7):
                        chunk = wa_pool.tile([128, 8, 128], dt.bfloat16, tag="wa_c", name="wa_c")
                        r0 = (m * 56 + kc * 8) * 128
                        nc.sync.dma_start(
                            chunk[:],
                            wa[r0:r0 + 1024, :].rearrange("(p a) f -> p a f", a=8),
                        )
                        for k8 in range(8):
                            k = kc * 8 + k8
                            nc.tensor.matmul(
                                (psA if k % 2 == 0 else psB)[:mp, :],
                                chunk[:, k8, :mp],
                                h_all[:, k, :],
                                start=(k < 2),
                                stop=(k >= 54),
                                skip_group_check=True,
                            )
                        if kc == 0 and pending_ss is not None:
                            # previous group's square-sum MM, emitted behind
                            # this group's first chunk so the PE never stalls
                            x2p, ssp, fir, las, was_m = pending_ss
                            nc.tensor.matmul(ssp[:], ones_b[:], x2p[:],
                                             start=fir, stop=las,
                                             skip_group_check=True)
                            pending_ss = None
                            if was_m == 15:
                                emit_kv_tail()
                    if m == 16:
                        nc.scalar.copy(kp_raw[:], psA[:64, :])
                        nc.vector.tensor_add(kp_raw[:], kp_raw[:], psB[:64, :])
                        # move the x2 half to base partition 0 for the DVE ops
                        nc.sync.dma_start(kp2[:], kp_raw[32:64, :])
                        kr1 = agt_pool.tile([32, TSH], dt.bfloat16, tag="kr1", name="kr1")
                        kr2 = agt_pool.tile([32, TSH], dt.bfloat16, tag="kr2", name="kr2")
                        t1 = x2_pool.tile([32, TSH], dt.bfloat16, tag="t1", name="t1")
                        t2 = x2_pool.tile([32, TSH], dt.bfloat16, tag="t2", name="t2")
                        nc.vector.tensor_mul(t1[:], kp_raw[0:32, :], csc_s[:])
                        nc.vector.tensor_mul(t2[:], kp2[:], csn_s[:])
                        nc.vector.tensor_sub(kr1[:], t1[:], t2[:])
                        t3 = x2_pool.tile([32, TSH], dt.bfloat16, tag="t1", name="t1")
                        t4 = x2_pool.tile([32, TSH], dt.bfloat16, tag="t2", name="t2")
                        nc.vector.tensor_mul(t3[:], kp_raw[0:32, :], csn_s[:])
                        nc.vector.tensor_mul(t4[:], kp2[:], csc_s[:])
                        nc.vector.tensor_add(kr2[:], t3[:], t4[:])
                        nc.sync.dma_start(agq_in[0][0:32, 4, :], kr1[:])
                        nc.sync.dma_start(agq_in[0][32:64, 4, :], kr2[:])
                    elif m < 12:
                        # q group: raw bf16 evict straight to the AG buffer
                        agt = agt_pool.tile([128, TSH], dt.bfloat16, tag="agt", name="agt")
                        nc.vector.tensor_copy(agt[:], psA[:])
                        nc.vector.tensor_add(agt[:], agt[:], psB[:])
                        blk = 5 + m if m < 4 else m % 4
                        nc.sync.dma_start(agq_in[m // 4][:, blk, :], agt[:])
                        x2 = x2_pool.tile([128, TSH], dt.bfloat16, tag="x2", name="x2")
                        nc.vector.tensor_mul(x2[:], agt[:], agt[:])
                        pending_ss = (x2, ss_q, m == 0, m == 11, m)
                        if m == 3 or m == 7:
                            nc.gpsimd.collective_compute(
                                "AllGather", mybir.AluOpType.bypass,
                                replica_groups=rg,
                                ins=[agq_in[m // 4].opt()],
                                outs=[agq_out[m // 4].opt()])
                    else:
                        nc.scalar.copy(qkv[m - 12][:], psA[:])
                        nc.vector.tensor_add(qkv[m - 12][:], qkv[m - 12][:], psB[:])
                        x2 = x2_pool.tile([128, TSH], dt.bfloat16, tag="x2", name="x2")
                        nc.vector.tensor_mul(x2[:], qkv[m - 12][:], qkv[m - 12][:])
                        pending_ss = (x2, ss_kv, m == 12, m == 15, m)

                # flush last q square-sum; build rstd row, launch q part2 AG
                x2p, ssp, fir, las, _ = pending_ss
                nc.tensor.matmul(ssp[:], ones_b[:], x2p[:], start=fir, stop=las,
                                 skip_group_check=True)
                bc_q = rstd_bcast(ss_q, QLR, "q")
                brs = agt_pool.tile([128, TSH], dt.bfloat16, tag="brs", name="brs")
                nc.vector.tensor_copy(brs[:], bc_q[:])
                nc.sync.dma_start(agq_in[2][:, 4, :], brs[:])
                nc.gpsimd.collective_compute(
                    "AllGather", mybir.AluOpType.bypass, replica_groups=rg,
                    ins=[agq_in[2].opt()], outs=[agq_out[2].opt()])

            # B/C weights + constants, behind phase A's DMA stream
            nc.scalar.dma_start(wqb_t[:], wqb.rearrange("p (a f) -> p a f", a=12))
            nc.scalar.dma_start(wkvb_t[:], wkvb.rearrange("p (a f) -> p a f", a=4))
            nc.scalar.dma_start(mask_sb[:], mask1[:, :])
            nc.scalar.dma_start(csc_f[:], cs_full[0:32, :])
            nc.scalar.dma_start(csn_f[:], cs_full[32:64, :])

            # ============================================================
            # Phase B: Q^T first (its AGs land first), then K^T / V
            # ============================================================
            bpools = contextlib.ExitStack()
            with bpools:
                act = bpools.enter_context(tc.tile_pool(name="act", bufs=1))
                pbp = contextlib.ExitStack()
                agq_pool = pbp.enter_context(tc.tile_pool(name="agq", bufs=4))
                agkv_pool = pbp.enter_context(tc.tile_pool(name="agkv", bufs=4))
                rsb_pool = pbp.enter_context(tc.tile_pool(name="rsb", bufs=4))
                xx_pool = pbp.enter_context(tc.tile_pool(name="xx", bufs=3))
                rt_pool = pbp.enter_context(tc.tile_pool(name="rt", bufs=1))
                ps_q = pbp.enter_context(tc.tile_pool(name="ps_q", bufs=4, space="PSUM"))
                ps_kt = pbp.enter_context(tc.tile_pool(name="ps_kt", bufs=2, space="PSUM"))
                ps_v = pbp.enter_context(tc.tile_pool(name="ps_v", bufs=2, space="PSUM"))

                qt_n = [act.tile([128, T], dt.bfloat16, tag=f"qtn{h}", name=f"qtn{h}")
                        for h in range(HPC)]
                qraw = act.tile([128, T], dt.bfloat16, tag="qraw", name="qraw")
                qt_r = [act.tile([64, T], dt.bfloat16, tag=f"qtr{h}", name=f"qtr{h}")
                        for h in range(HPC)]
                kt_n = [act.tile([128, T], dt.bfloat16, tag=f"ktn{h}", name=f"ktn{h}")
                        for h in range(HPC)]
                kpe_t = act.tile([64, T], dt.bfloat16, tag="kpet", name="kpet")
                # v2_t[ti]: both heads' V for token tile ti, cols h*128..
                v2_t = [act.tile([128, 2 * D_V], dt.bfloat16, tag=f"v{i}", name=f"v{i}")
                        for i in range(16)]

                # --- all B loads issued up front: kv (lands first), then
                # aq parts 0/1, then part 2 last so nothing it waits on can
                # head-of-line-block earlier loads; rstd/kpe on the ACT ring
                akv_all = []
                aq_all = []
                rstd_all = []
                for rp in range(4):
                    akv = agkv_pool.tile([128, 4, 512], dt.bfloat16, tag="akv", name="akv")
                    akv_all.append(akv)
                    for s in range(2):
                        r = 2 * rp + s
                        nc.sync.dma_start(
                            akv[:, :, s * TSH:(s + 1) * TSH],
                            agq_out[0][r * 128:(r + 1) * 128, 0:4, :])
                for rp in range(4):
                    aq_all.append(agq_pool.tile([128, 12, 512], dt.bfloat16,
                                                tag="aq", name="aq"))
                    rstd_all.append(rsb_pool.tile([128, 512], dt.bfloat16,
                                                  tag="rsb", name="rsb"))
                for p3 in range(3):
                    for rp in range(4):
                        for s in range(2):
                            r = 2 * rp + s
                            ssl = slice(s * TSH, (s + 1) * TSH)
                            srcblk = 5 if p3 == 0 else 0
                            nc.sync.dma_start(
                                aq_all[rp][:, p3 * 4:p3 * 4 + 4, ssl],
                                agq_out[p3][r * 128:(r + 1) * 128,
                                            srcblk:srcblk + 4, :])
                for rp in range(4):
                    for s in range(2):
                        r = 2 * rp + s
                        ssl = slice(s * TSH, (s + 1) * TSH)
                        nc.scalar.dma_start(
                            rstd_all[rp][:, ssl],
                            agq_out[2][r * 128:(r + 1) * 128, 4, :])
                        nc.scalar.dma_start(
                            kpe_t[:, r * TSH:(r + 1) * TSH],
                            agq_out[0][r * 128:r * 128 + 64, 4, :])

                # --- KV path first (its AG lands first) ---
                for rp in range(4):
                    tsl = slice(rp * 512, (rp + 1) * 512)
                    akv = akv_all[rp]
                    pk0 = ps_kt.tile([128, 512], dt.float32, tag="pkv", name="pkv")
                    pk1 = ps_kt.tile([128, 512], dt.float32, tag="pkv", name="pkv")
                    for kk in range(4):
                        nc.tensor.matmul(
                            pk0[:], wkvb_t[:, kk, 0:128], akv[:, kk, :],
                            start=(kk == 0), stop=(kk == 3),
                            skip_group_check=True)
                        nc.tensor.matmul(
                            pk1[:], wkvb_t[:, kk, 128:256], akv[:, kk, :],
                            start=(kk == 0), stop=(kk == 3),
                            skip_group_check=True)
                    nc.vector.tensor_copy(kt_n[0][:, tsl], pk0[:])
                    nc.vector.tensor_copy(kt_n[1][:, tsl], pk1[:])
                    for pr in range(2):
                        pv0 = ps_v.tile([128, 2 * D_V], dt.float32, tag="pv_b", name="pv_b")
                        pv1 = ps_v.tile([128, 2 * D_V], dt.float32, tag="pv_b", name="pv_b")
                        for kk in range(4):
                            for s4, pv in ((2 * pr, pv0), (2 * pr + 1, pv1)):
                                nc.tensor.matmul(
                                    pv[:],
                                    akv[:, kk, s4 * 128:(s4 + 1) * 128],
                                    wkvb_t[:, kk, 256:512],
                                    start=(kk == 0), stop=(kk == 3),
                                    skip_group_check=True)
                        nc.vector.tensor_copy(v2_t[4 * rp + 2 * pr][:], pv0[:])
                        nc.vector.tensor_copy(v2_t[4 * rp + 2 * pr + 1][:], pv1[:])

                # --- Q path ---
                for rp in range(4):
                    tsl = slice(rp * 512, (rp + 1) * 512)
                    aq = aq_all[rp]
                    rstd_bc = rstd_all[rp]
                    pnA = ps_q.tile([128, 512], dt.float32, tag="pq", name="pq")
                    pnB = ps_q.tile([128, 512], dt.float32, tag="pq", name="pq")
                    pR = ps_q.tile([128, 512], dt.float32, tag="pq", name="pq")
                    for kq in range(12):
                        nc.tensor.matmul(
                            pnA[:], wqb_t[:, kq, 0:128], aq[:, kq, :],
                            start=(kq == 0), stop=(kq == 11), skip_group_check=True)
                        nc.tensor.matmul(
                            pnB[:], wqb_t[:, kq, 128:256], aq[:, kq, :],
                            start=(kq == 0), stop=(kq == 11), skip_group_check=True)
                        nc.tensor.matmul(
                            pR[:], wqb_t[:, kq, 256:384], aq[:, kq, :],
                            start=(kq == 0), stop=(kq == 11), skip_group_check=True)
                    nc.vector.tensor_mul(qt_n[0][:, tsl], pnA[:], rstd_bc[:])
                    nc.vector.tensor_mul(qt_n[1][:, tsl], pnB[:], rstd_bc[:])
                    nc.vector.tensor_mul(qraw[:, tsl], pR[:], rstd_bc[:])
                    # rope: qraw rows [h0ev, h0od, h1ev, h1od] (32 each)
                    x2h0 = xx_pool.tile([32, 512], dt.bfloat16, tag="xx", name="xx")
                    x1h1 = xx_pool.tile([32, 512], dt.bfloat16, tag="xx", name="xx")
                    x2h1 = xx_pool.tile([32, 512], dt.bfloat16, tag="xx", name="xx")
                    nc.sync.dma_start(x2h0[:], qraw[32:64, tsl])
                    nc.sync.dma_start(x1h1[:], qraw[64:96, tsl])
                    nc.sync.dma_start(x2h1[:], qraw[96:128, tsl])
                    for h, (x1, x2) in enumerate([(qraw[0:32, tsl], x2h0[:]),
                                                  (x1h1[:], x2h1[:])]):
                        cc = csc_f[:, tsl]
                        ss = csn_f[:, tsl]
                        t1 = rt_pool.tile([32, 512], dt.bfloat16, tag="qt1", name="qt1")
                        t2 = rt_pool.tile([32, 512], dt.bfloat16, tag="qt2", name="qt2")
                        nc.vector.tensor_mul(t1[:], x1, cc)
                        nc.vector.tensor_mul(t2[:], x2, ss)
                        nc.vector.tensor_sub(qt_r[h][0:32, tsl], t1[:], t2[:])
                        t3 = rt_pool.tile([32, 512], dt.bfloat16, tag="qt1", name="qt1")
                        t4 = rt_pool.tile([32, 512], dt.bfloat16, tag="qt2", name="qt2")
                        nc.vector.tensor_mul(t3[:], x1, ss)
                        nc.vector.tensor_mul(t4[:], x2, cc)
                        r2t = rt_pool.tile([32, 512], dt.bfloat16, tag="r2t", name="r2t")
                        nc.vector.tensor_add(r2t[:], t3[:], t4[:])
                        nc.sync.dma_start(qt_r[h][32:64, tsl], r2t[:])

                pbp.close()

                # ========================================================
                # Phase C: attention, software-pipelined (lookahead 2)
                # ========================================================
                with contextlib.ExitStack() as pc:
                    pt_pool = pc.enter_context(tc.tile_pool(name="pt", bufs=3))
                    sm_pool = pc.enter_context(tc.tile_pool(name="sm", bufs=2))
                    acc_pool = pc.enter_context(tc.tile_pool(name="acc", bufs=2))
                    ps_s = pc.enter_context(tc.tile_pool(name="ps_s", bufs=4, space="PSUM"))
                    ps_pv = pc.enter_context(tc.tile_pool(name="ps_pv", bufs=2, space="PSUM"))
                    ps_l = pc.enter_context(tc.tile_pool(name="ps_l", bufs=1, space="PSUM"))
                    ps_b = pc.enter_context(tc.tile_pool(name="ps_b", bufs=1, space="PSUM"))

                    attn = [act.tile([128, T], dt.bfloat16, tag=f"attn{h}",
                                     name=f"attn{h}") for h in range(HPC)]
                    # head-0 strips live in a top-level pool so their loads
                    # can be issued as soon as the first AG2 completes; the
                    # o_proj weights load here too (phase-B pools just freed)
                    stripsA = [epool.tile([128, 2, T], dt.bfloat16, tag=f"sa{p}",
                                          name=f"sa{p}") for p in range(4)]
                    wo_t = epool.tile([128, 16, WO_COLS], dt.bfloat16, tag="wo",
                                      name="wo")
                    nc.scalar.dma_start(wo_t[:], wo.rearrange("p (a f) -> p a f", a=16))

                    steps = [(h, j, ki) for h in range(HPC) for j in range(4)
                             for ki in range(4 * j + 4)]

                    def emit_s1(h, j, ki):
                        # nope-dim score matmul (start of the 192-dim pair)
                        off = max(0, (ki - 4 * j) * 128)
                        w = 512 - off
                        qs0 = j * 512 + off
                        ksl = slice(ki * 128, (ki + 1) * 128)
                        ps = ps_s.tile([128, 512], dt.float32, tag="ps", name="ps")
                        nc.tensor.matmul(
                            ps[:, off:], kt_n[h][:, ksl], qt_n[h][:, qs0:qs0 + w],
                            start=True, stop=False, skip_group_check=True)
                        return ps

                    def emit_s2(h, j, ki, ps):
                        # rope-dim score matmul; staggered one step behind
                        # s1 so consecutive matmuls alternate PSUM banks
                        off = max(0, (ki - 4 * j) * 128)
                        w = 512 - off
                        qs0 = j * 512 + off
                        ksl = slice(ki * 128, (ki + 1) * 128)
                        nc.tensor.matmul(
                            ps[:, off:], kpe_t[:, ksl], qt_r[h][:, qs0:qs0 + w],
                            start=False, stop=True, skip_group_check=True)

                    pend = {}
                    pend[steps[0]] = emit_s1(*steps[0])
                    pend[steps[1]] = emit_s1(*steps[1])
                    emit_s2(*steps[0], pend[steps[0]])
                    cur = {0: {}, 1: {}}
                    for idx, (h, j, ki) in enumerate(steps):
                        nk = 4 * j + 4
                        qsl = slice(j * 512, (j + 1) * 512)
                        if ki == 0:
                            cur[h]['ppv'] = ps_pv.tile([128, 512], dt.float32,
                                                       tag="ppv", name="ppv")
                            cur[h]['acc'] = acc_pool.tile([128, 512], dt.bfloat16,
                                                          tag="acc", name="acc")
                        if idx + 2 < len(steps):
                            pend[steps[idx + 2]] = emit_s1(*steps[idx + 2])
                        if idx + 1 < len(steps):
                            emit_s2(*steps[idx + 1], pend[steps[idx + 1]])
                        ps = pend.pop((h, j, ki))
                        off = max(0, (ki - 4 * j) * 128)
                        if ki >= 4 * j:
                            nc.vector.tensor_add(
                                ps[:, off:off + 128], ps[:, off:off + 128],
                                mask_sb[:])
                        pt = pt_pool.tile([128, 512], dt.bfloat16, tag="pt", name="pt")
                        nc.scalar.activation(pt[:, off:], ps[:, off:], AF.Exp)
                        # denominator partial sums on DVE (saves a PE matmul
                        # per tile); reduced across partitions once per j
                        if ki == 0:
                            nc.vector.tensor_copy(cur[h]['acc'][:], pt[:])
                        else:
                            nc.vector.tensor_add(cur[h]['acc'][:, off:],
                                                 cur[h]['acc'][:, off:], pt[:, off:])
                        nc.tensor.matmul(
                            cur[h]['ppv'][:, off:], v2_t[ki][:, h * D_V:(h + 1) * D_V],
                            pt[:, off:],
                            start=(ki == 0), stop=(ki == nk - 1),
                            skip_group_check=True)
                        if ki == nk - 1:
                            # l = colsum(acc); 1/l via exp(-ln(l)) on ACT
                            # (DVE reciprocal is ~6ns/elem; ln+exp is ~1.7x
                            # faster and both live in the loaded table set)
                            pl = ps_l.tile([1, 512], dt.float32, tag="pl", name="pl")
                            nc.tensor.matmul(pl[:], ones_b[:], cur[h]['acc'][:],
                                             start=True, stop=True,
                                             skip_group_check=True)
                            pls = sm_pool.tile([1, 512], dt.float32, tag="pls", name="pls")
                            nc.scalar.copy(pls[:], pl[:])
                            pb = ps_b.tile([128, 512], dt.float32, tag="pb", name="pb")
                            nc.tensor.matmul(pb[:], ones_f[:], pls[:],
                                             start=True, stop=True,
                                             skip_group_check=True)
                            # NOTE: keep everything here off ScalarE's Exp
                            # table set — Ln/Reciprocal live in different
                            # sets and each switch costs ~2.7us + disrupts
                            # the exp pipeline (measured 9 table loads/C)
                            rb = sm_pool.tile([128, 512], dt.float32, tag="rb", name="rb")
                            nc.vector.reciprocal(rb[:], pb[:])
                            nc.vector.tensor_mul(attn[h][:, qsl], cur[h]['ppv'][:], rb[:])
                            if j == 3:
                                nc.sync.dma_start(ag2_in[h][:], attn[h][:])
                                nc.gpsimd.collective_compute(
                                    "AllGather", mybir.AluOpType.bypass,
                                    replica_groups=rg,
                                    ins=[ag2_in[h].opt()], outs=[ag2_out[h].opt()])
                                if h == 0:
                                    for p in range(4):
                                        for z in range(2):
                                            r = 2 * p + z
                                            nc.sync.dma_start(
                                                stripsA[p][:, z, :],
                                                ag2_out[0][r * 128:(r + 1) * 128, :])

            # ============================================================
            # Phase E: o_proj column slice, two K-waves (head0 then head1)
            # ============================================================
            with contextlib.ExitStack() as pe:
                ao_pool = pe.enter_context(tc.tile_pool(name="ao", bufs=1))
                oa_pool = pe.enter_context(tc.tile_pool(name="oa", bufs=1))
                oo_pool = pe.enter_context(tc.tile_pool(name="oo", bufs=3))
                ps_o = pe.enter_context(tc.tile_pool(name="ps_o", bufs=8, space="PSUM"))

                # strips paired [128, 2, T]; wo blocks host-reordered
                # [0,2,..,14, 1,3,..,15] (wave A then wave B); stripsA loads
                # were issued at the first AG2 trigger in phase C, stripsB
                # loads here overlap wave A's matmuls
                stripsB = [ao_pool.tile([128, 2, T], dt.bfloat16, tag=f"sb{p}",
                                        name=f"sb{p}") for p in range(4)]
                otA = [oa_pool.tile([128, T], dt.bfloat16, tag=f"oa{mt}", name=f"oa{mt}")
                       for mt in range(7)]
                for p in range(4):
                    for z in range(2):
                        r = 2 * p + z
                        nc.sync.dma_start(stripsB[p][:, z, :],
                                          ag2_out[1][r * 128:(r + 1) * 128, :])
                # wave A: heads 0,2,..,14 (wo blocks 0..7)
                for mt in range(7):
                    msl = slice(mt * 128, (mt + 1) * 128)
                    pes = [ps_o.tile([128, 512], dt.float32, tag="po", name="po")
                           for _ in range(4)]
                    for p in range(4):
                        for z in range(2):
                            for n in range(4):
                                nc.tensor.matmul(
                                    pes[n][:], wo_t[:, 2 * p + z, msl],
                                    stripsA[p][:, z, n * 512:(n + 1) * 512],
                                    start=(p == 0 and z == 0),
                                    stop=(p == 3 and z == 1),
                                    skip_group_check=True)
                    for n in range(4):
                        nc.scalar.copy(otA[mt][:, n * 512:(n + 1) * 512], pes[n][:])
                # wave B: heads 1,3,..,15 (wo blocks 8..15), add wave A partial
                for mt in range(7):
                    msl = slice(mt * 128, (mt + 1) * 128)
                    pes = [ps_o.tile([128, 512], dt.float32, tag="po", name="po")
                           for _ in range(4)]
                    for p in range(4):
                        for z in range(2):
                            for n in range(4):
                                nc.tensor.matmul(
                                    pes[n][:], wo_t[:, 8 + 2 * p + z, msl],
                                    stripsB[p][:, z, n * 512:(n + 1) * 512],
                                    start=(p == 0 and z == 0),
                                    stop=(p == 3 and z == 1),
                                    skip_group_check=True)
                    ot = oo_pool.tile([128, T], dt.float32, tag="ot", name="ot")
                    for n in range(4):
                        nc.vector.tensor_add(
                            ot[:, n * 512:(n + 1) * 512], pes[n][:],
                            otA[mt][:, n * 512:(n + 1) * 512])
                    nc.sync.dma_start(out[msl, :], ot[:])


def _prep_inputs(hidden_states, positions, W_qkv_a, gamma_q, W_qb, gamma_kv,
                 W_kvb, W_o):
    f32 = np.float32
    perm = np.concatenate([np.arange(0, D_ROPE, 2), np.arange(1, D_ROPE, 2)])
    scale = np.float32(D_QK ** -0.5)

    def pmajor(w, nblk):
        # [nblk*128, F] -> [128, nblk*F] so a partition line is contiguous
        F = w.shape[1]
        return np.ascontiguousarray(
            w.reshape(nblk, 128, F).transpose(1, 0, 2).reshape(128, nblk * F)
        ).astype(BF16)

    # A-projection weights: de-interleave k_pe output cols, block layout
    Wa = np.asarray(W_qkv_a, f32).copy()
    Wa[:, QLR + KVLR:] = Wa[:, QLR + KVLR:][:, perm]
    Wa = np.concatenate([Wa, np.zeros((H, 64), f32)], axis=1)  # pad 2112->2176
    wa_b = (
        Wa.reshape(7, 8, 128, 17, 128)   # [kc, k8, p, m, f]
        .transpose(3, 0, 2, 1, 4)        # [m, kc, p, k8, f]
        .reshape(17 * 56 * 128, 128)
        .astype(BF16)
    )

    # q_b weights: fold gamma_q and score scale, de-interleave rope cols,
    # pack per-core as [h0 nope | h1 nope | h0 ev | h0 od | h1 ev | h1 od]
    Wqb = (np.asarray(W_qb, f32) * np.asarray(gamma_q, f32)[:, None] * scale)
    Wqb = Wqb.reshape(QLR, NH, D_QK)
    Wqb_n = Wqb[:, :, :D_NOPE]
    Wqb_r = Wqb[:, :, D_NOPE:][:, :, perm]   # [QLR, NH, 64] ev|od

    # kv_b weights: fold gamma_kv
    Wkvb = (np.asarray(W_kvb, f32) * np.asarray(gamma_kv, f32)[:, None])
    Wkvb = Wkvb.reshape(KVLR, NH, D_NOPE + D_V)

    Wo = np.asarray(W_o, f32)

    hTf = np.asarray(hidden_states, f32).T.astype(BF16)  # [H, T]

    pos = np.asarray(positions, f32)
    inv_freq = 1.0 / (THETA ** (np.arange(D_ROPE // 2, dtype=f32) / (D_ROPE // 2)))
    freqs = pos[:, None] * inv_freq[None, :]          # [T, 32]
    cos = np.cos(freqs).astype(f32).T                 # [32, T]
    sin = np.sin(freqs).astype(f32).T
    cs = np.concatenate([cos, sin], axis=0)           # [64, T]

    kk = np.arange(128)[:, None]
    qq = np.arange(128)[None, :]
    mask1 = np.where(qq < kk, np.float32(NEG), np.float32(0.0)).astype(f32)

    in_maps = []
    for c in range(NCORES):
        h0, h1 = 2 * c, 2 * c + 1
        wqb_c = np.concatenate(
            [Wqb_n[:, h0, :], Wqb_n[:, h1, :], Wqb_r[:, h0, :], Wqb_r[:, h1, :]],
            axis=1)                                  # [QLR, 384]
        wkvb_c = np.concatenate(
            [Wkvb[:, h0, :D_NOPE], Wkvb[:, h1, :D_NOPE],
             Wkvb[:, h0, D_NOPE:], Wkvb[:, h1, D_NOPE:]], axis=1)  # [KVLR, 512]
        hsh = np.ascontiguousarray(hTf[:, c * TSH:(c + 1) * TSH])  # [H, TSH]
        hsh = hsh.reshape(56, 128, TSH).transpose(1, 0, 2).reshape(128, 56 * TSH)
        # wo: fp8, head blocks reordered [0,2,..,14, 1,3,..,15] so the
        # DoubleRow pairs within each AG2 wave are adjacent
        wo_c = Wo[:, c * WO_COLS:(c + 1) * WO_COLS].reshape(16, 128, WO_COLS)
        wo_c = wo_c[list(range(0, 16, 2)) + list(range(1, 16, 2))]
        wo_c = (wo_c.transpose(1, 0, 2).reshape(128, 16 * WO_COLS)
                .astype(BF16))
        in_maps.append({
            "hT": np.ascontiguousarray(hsh),
            "wa": wa_b,
            "wqb": pmajor(wqb_c, 12),
            "wkvb": pmajor(wkvb_c, 4),
            "wo": np.ascontiguousarray(wo_c),
            "cs_sh": np.ascontiguousarray(cs[:, c * TSH:(c + 1) * TSH]).astype(BF16),
            "cs_full": cs.astype(BF16),
            "mask1": mask1,
        })
    return in_maps


def kernel(hidden_states, positions, W_qkv_a, gamma_q, W_qb, gamma_kv, W_kvb,
           W_o, _trace=False, _tmpdir=None):
    from concourse.bass_utils import run_bass_kernel_spmd

    if "nc" not in _CACHE:
        _CACHE["nc"] = _build()
    nc = _CACHE["nc"]

    in_maps = _prep_inputs(hidden_states, positions, W_qkv_a, gamma_q, W_qb,
                           gamma_kv, W_kvb, W_o)
    res = run_bass_kernel_spmd(nc, in_maps, list(range(NCORES)), trace=_trace,
                               tmpdir=_tmpdir)
    _CACHE["last_result"] = res
    out = np.concatenate(
        [res.results[c]["out"].T for c in range(NCORES)], axis=1)
    return out.astype(np.float32)
